# revision 16
# baseline (speedup 1.0000x reference)
"""GCN encoder (2x GCNConv + GraphNorm + ReLU + mean-pool) on 8 trn2 cores.

Strategy: graph-aligned node sharding across 8 cores (batch is sorted, so
each core owns a contiguous run of whole graphs -> GraphNorm and pooling are
fully shard-local). Edges are assigned to the core that owns their dst node.
Each core:
  - computes hp = dinv * (x @ W1) for its shard (PE), keeping hp resident in
    SBUF in partition-major chunk layout [128, n_chunks, F],
  - stores hp to DRAM in one contiguous DMA and AllGathers it so every core
    holds the full (partition-major) node table,
  - aggregates messages per 128-dst window: dma_gather streams message rows
    in 1024-row calls (the int16 index limit splits the table into 4
    quarters; a ring of [128, 8, F] tiles per quarter), one DVE is_equal
    builds all one-hot selection matrices of a window at once, C PE matmuls
    accumulate in PSUM, the self-loop is folded in with a DVE add into the
    SBUF-resident agg tile,
  - applies symmetric-norm scaling, bias, GraphNorm (stats via St matmuls,
    per-node A/B coefficients fetched with dma_gather from a [G, 2F] DRAM
    table), ReLU,
  - repeats for layer 2, then computes per-graph mean pooling (PE).
Host side: numpy preprocessing (degrees, sharding, window schedule) + final
assembly of the [128, 64] output.
"""
import numpy as np

import concourse.bass as bass
import concourse.bacc as bacc
import concourse.mybir as mybir
import concourse.tile as tile
from concourse.bass_utils import run_bass_kernel_spmd

dt = mybir.dt

NCORES = 8
NUM_NODES = 100000
NUM_EDGES = 1600000
NUM_GRAPHS = 128
IN_DIM, HID_DIM, EMB_DIM = 128, 64, 64
EPS = 1e-5
GMAX = 32          # max graphs per core (128/8 = 16 avg)
P = 128
QN = 4             # table quarters (int16 gather index range)
CBLK = 8           # blocks per gather call (1024-index ucode cap)

_CACHE = {}


def _shard_plan(batch):
    """Graph-aligned contiguous node shards."""
    counts = np.bincount(batch, minlength=NUM_GRAPHS)
    starts = np.concatenate([[0], np.cumsum(counts)])  # [G+1]
    bounds = [0]
    for k in range(1, NCORES):
        target = k * NUM_NODES // NCORES
        g = int(np.argmin(np.abs(starts - target)))
        bounds.append(int(starts[g]))
    bounds.append(NUM_NODES)
    node_bounds = np.array(bounds)
    graph_bounds = [int(np.searchsorted(starts, b)) for b in bounds]
    return node_bounds, starts, graph_bounds


def _plan(batch, src, dst):
    """Shared host plan: shards, row remap, per-quarter block caps."""
    node_bounds, gstarts, graph_bounds = _shard_plan(batch)
    n_shard = int(np.max(node_bounds[1:] - node_bounds[:-1]))
    # +P guarantees every core has >=1 all-zero pad row (gather target for
    # padding slots in the edge schedule)
    n_shard = ((n_shard + P) // P) * P
    n_chunks = n_shard // P

    # remap node -> partition-major row in hp_full: (core*128 + p)*n_chunks + c
    core_of = np.searchsorted(node_bounds, np.arange(NUM_NODES),
                              side="right") - 1
    local = np.arange(NUM_NODES) - node_bounds[core_of]
    remap = ((core_of * P + local % P) * n_chunks + local // P)

    NROWS = NCORES * P * n_chunks
    RQ = NROWS // QN

    # per-quarter window block caps (global max over cores&windows)
    ecore = core_of[dst]
    per_core = []
    cq = np.zeros(QN, np.int64)
    for k in range(NCORES):
        m = ecore == k
        sl = src[m]
        dl = dst[m] - node_bounds[k]
        rq = remap[sl] // RQ
        for q in range(QN):
            cnt = np.bincount((dl // P)[rq == q], minlength=n_chunks)
            cq[q] = max(cq[q], int(np.max((cnt + P - 1) // P)))
        per_core.append((sl, dl, rq))
    return (node_bounds, graph_bounds, n_shard, n_chunks, remap, NROWS, RQ,
            per_core, cq)


def _wrap16(idx_flat):
    """[N] int -> [128, N/16] int16 in dma_gather wrapped layout
    (index i at [i%16, i//16], replicated x8 over partitions)."""
    n = idx_flat.shape[0]
    assert n % 16 == 0
    w = np.zeros((16, n // 16), np.int16)
    w[np.arange(n) % 16, np.arange(n) // 16] = idx_flat.astype(np.int16)
    return np.tile(w, (8, 1))


def _build_program(n_shard, n_chunks, cq):
    """Bass program; identical across cores. cq = per-quarter block caps."""
    nc = bacc.Bacc("TRN2", target_bir_lowering=False, debug=False,
                   num_devices=NCORES, detect_race_conditions=False,
                   num_swdge_queues=4)

    N_S = n_shard
    cq = [int(v) for v in cq]
    C = sum(cq)                    # blocks per window
    qoff = [0]
    for v in cq:
        qoff.append(qoff[-1] + v)
    NROWS = NCORES * P * n_chunks  # partition-major row table size
    RQ = NROWS // QN
    assert NROWS % QN == 0 and RQ <= 32767
    F = HID_DIM  # 64 (== EMB_DIM)
    # per-quarter gather-call schedule: quarter stream has n_chunks*cq[q]
    # blocks, cut into CBLK-block calls
    qblocks = [n_chunks * cq[q] for q in range(QN)]
    qcalls = [(qb + CBLK - 1) // CBLK for qb in qblocks]
    # wrapped-int16 column base per quarter (8 cols per block)
    qcolbase = [0]
    for qb in qblocks:
        qcolbase.append(qcolbase[-1] + qb * 8)
    EIC = qcolbase[-1]

    # ---------------- inputs ----------------
    xkT = nc.dram_tensor("xkT", [IN_DIM, N_S], dt.float32, kind="ExternalInput")
    W1 = nc.dram_tensor("W1", [IN_DIM, HID_DIM], dt.float32, kind="ExternalInput")
    W2 = nc.dram_tensor("W2", [HID_DIM, EMB_DIM], dt.float32, kind="ExternalInput")
    b1r = nc.dram_tensor("b1r", [P, 3 * HID_DIM], dt.float32, kind="ExternalInput")
    b2r = nc.dram_tensor("b2r", [P, 3 * EMB_DIM], dt.float32, kind="ExternalInput")
    dinvk = nc.dram_tensor("dinvk", [P, n_chunks], dt.float32, kind="ExternalInput")
    ident = nc.dram_tensor("ident", [P, P], dt.float32, kind="ExternalInput")
    gnp1 = nc.dram_tensor("gnp1", [GMAX, 3 * HID_DIM], dt.float32,
                          kind="ExternalInput")  # [alpha|weight|bias]
    gnp2 = nc.dram_tensor("gnp2", [GMAX, 3 * EMB_DIM], dt.float32,
                          kind="ExternalInput")
    cntinv = nc.dram_tensor("cntinv", [GMAX, 1], dt.float32, kind="ExternalInput")
    S_in = nc.dram_tensor("S_in", [n_chunks, P, GMAX], dt.float32,
                          kind="ExternalInput")
    dstv1 = nc.dram_tensor("dstv1", [P, n_chunks * C], dt.float32,
                           kind="ExternalInput")
    eidx_in = nc.dram_tensor("eidx_in", [P, EIC], dt.int16, kind="ExternalInput")
    gb16_in = nc.dram_tensor("gb16_in", [P, n_chunks * 8], dt.int16,
                             kind="ExternalInput")
    iotw_in = nc.dram_tensor("iotw_in", [P, C * P], dt.float32,
                             kind="ExternalInput")

    pool_out = nc.dram_tensor("pool_out", [GMAX, EMB_DIM], dt.float32,
                              kind="ExternalOutput")

    # ---------------- internal DRAM ----------------
    hp_loc1 = nc.dram_tensor("hp_loc1", [P, n_chunks, F], dt.float32)
    hp_loc2 = nc.dram_tensor("hp_loc2", [P, n_chunks, F], dt.float32)
    hp_full1 = nc.dram_tensor("hp_full1", [NROWS, F], dt.float32,
                              addr_space="Shared")
    hp_full2 = nc.dram_tensor("hp_full2", [NROWS, F], dt.float32,
                              addr_space="Shared")
    AB1d = nc.dram_tensor("AB1d", [GMAX, 2 * F], dt.float32)
    AB2d = nc.dram_tensor("AB2d", [GMAX, 2 * F], dt.float32)

    GXD = 9                       # dense-1 chunks per load
    assert n_chunks % GXD == 0

    with tile.TileContext(nc) as tc:
        with (
            tc.tile_pool(name="const", bufs=1) as cpool,
            tc.tile_pool(name="xload", bufs=2) as xpool,
            tc.tile_pool(name="work", bufs=4) as wpool,
            tc.tile_pool(name="msg", bufs=1) as mpool,
            tc.tile_pool(name="psmm", bufs=2, space="PSUM") as psmm,
            tc.tile_pool(name="pstr", bufs=2, space="PSUM") as pstr,
            tc.tile_pool(name="psacc", bufs=1, space="PSUM") as psacc,
        ):
            idt = cpool.tile([P, P], dt.float32)
            nc.sync.dma_start(idt[:], ident[:])
            w1t = cpool.tile([IN_DIM, HID_DIM], dt.float32)
            nc.sync.dma_start(w1t[:], W1[:])
            w2t = cpool.tile([HID_DIM, EMB_DIM], dt.float32)
            nc.sync.dma_start(w2t[:], W2[:])
            dinvt = cpool.tile([P, n_chunks], dt.float32)
            nc.sync.dma_start(dinvt[:], dinvk[:])
            b1t = cpool.tile([P, 3 * HID_DIM], dt.float32)
            nc.sync.dma_start(b1t[:], b1r[:])
            b2t = cpool.tile([P, 3 * EMB_DIM], dt.float32)
            nc.sync.dma_start(b2t[:], b2r[:])
            St = cpool.tile([P, n_chunks, GMAX], dt.float32)
            nc.sync.dma_start(St[:], S_in.ap().rearrange("c p g -> p c g"))
            gnp1t = cpool.tile([GMAX, 3 * HID_DIM], dt.float32)
            nc.sync.dma_start(gnp1t[:], gnp1[:])
            gnp2t = cpool.tile([GMAX, 3 * EMB_DIM], dt.float32)
            nc.sync.dma_start(gnp2t[:], gnp2[:])
            cit = cpool.tile([GMAX, 1], dt.float32)
            nc.sync.dma_start(cit[:], cntinv[:])
            iotw = cpool.tile([P, C * P], dt.float32)
            nc.sync.dma_start(iotw[:], iotw_in[:])
            dvt = cpool.tile([P, n_chunks * C], dt.float32)
            nc.sync.dma_start(dvt[:], dstv1[:])
            ei16 = cpool.tile([P, EIC], dt.int16)
            nc.sync.dma_start(ei16[:], eidx_in[:])
            gb16 = cpool.tile([P, n_chunks * 8], dt.int16)
            nc.sync.dma_start(gb16[:], gb16_in[:])

            # SBUF residents
            hp_res = cpool.tile([P, n_chunks, F], dt.float32)
            agg_res = cpool.tile([P, n_chunks, F], dt.float32)

            def dense1():
                for pc in range(n_chunks // GXD):
                    xt = xpool.tile([IN_DIM, GXD * P], dt.float32, tag="xt")
                    nc.sync.dma_start(xt[:], xkT[:, pc * GXD * P:(pc + 1) * GXD * P])
                    for j in range(GXD):
                        c = pc * GXD + j
                        h_ps = psmm.tile([P, F], dt.float32, tag="mm")
                        nc.tensor.matmul(h_ps[:], lhsT=xt[:, j * P:(j + 1) * P],
                                         rhs=w1t[:], start=True, stop=True)
                        nc.vector.tensor_tensor(
                            out=hp_res[:, c, :], in0=h_ps[:],
                            in1=dinvt[:, c:c + 1].to_broadcast([P, F]),
                            op=mybir.AluOpType.mult)

            def dense2():
                for c in range(n_chunks):
                    tr_ps = pstr.tile([F, P], dt.float32, tag="tr")
                    nc.tensor.transpose(out=tr_ps[:], in_=agg_res[:, c, :],
                                        identity=idt[:])
                    xts = wpool.tile([F, P], dt.float32, tag="xts")
                    nc.scalar.activation(xts[:], tr_ps[:],
                                         mybir.ActivationFunctionType.Copy)
                    h_ps = psmm.tile([P, F], dt.float32, tag="mm")
                    nc.tensor.matmul(h_ps[:], lhsT=xts[:], rhs=w2t[:],
                                     start=True, stop=True)
                    nc.vector.tensor_tensor(
                        out=hp_res[:, c, :], in0=h_ps[:],
                        in1=dinvt[:, c:c + 1].to_broadcast([P, F]),
                        op=mybir.AluOpType.mult)

            def share(hp_loc, hp_full):
                nc.sync.dma_start(hp_loc[:], hp_res[:])
                nc.gpsimd.collective_compute(
                    "AllGather", mybir.AluOpType.bypass,
                    replica_groups=[list(range(NCORES))],
                    ins=[hp_loc.ap()], outs=[hp_full.ap()])

            def edge_phase(table_full):
                iotw3 = iotw[:].rearrange("p (c q) -> p c q", q=P)
                qtiles = [{} for _ in range(QN)]   # call idx -> tile
                issued = [0] * QN

                def issue(q):
                    t = issued[q]
                    nb = min(CBLK, qblocks[q] - t * CBLK)
                    mt = mpool.tile([P, CBLK, F], dt.float32,
                                    tag=f"msg{q}", bufs=3)
                    ecol = qcolbase[q] + t * CBLK * 8
                    nc.gpsimd.dma_gather(
                        out_ap=mt[:, 0:nb, :],
                        in_ap=table_full[q * RQ:(q + 1) * RQ, :],
                        idxs_ap=ei16[:, ecol:ecol + nb * 8],
                        num_idxs=nb * P, num_idxs_reg=nb * P,
                        elem_size=F, queue_num=q)
                    qtiles[q][t] = mt
                    issued[q] = t + 1

                for w in range(n_chunks):
                    for q in range(QN):
                        need = (cq[q] * (w + 1) + CBLK - 1) // CBLK
                        while issued[q] < min(need, qcalls[q]):
                            issue(q)
                    S = mpool.tile([P, C, P], dt.float32, tag="sel", bufs=2)
                    nc.vector.tensor_tensor(
                        out=S[:],
                        in0=dvt[:, w * C:(w + 1) * C].to_broadcast([P, C, P]),
                        in1=iotw3, op=mybir.AluOpType.is_equal)
                    ps = psmm.tile([P, F], dt.float32, tag="mm")
                    jj = 0
                    for q in range(QN):
                        for j in range(cq[q]):
                            b = w * cq[q] + j
                            mt = qtiles[q][b // CBLK]
                            nc.tensor.matmul(
                                ps[:], lhsT=S[:, jj, :],
                                rhs=mt[:, b % CBLK, :],
                                start=(jj == 0), stop=(jj == C - 1))
                            jj += 1
                    nc.vector.tensor_tensor(
                        out=agg_res[:, w, :], in0=ps[:],
                        in1=hp_res[:, w, :], op=mybir.AluOpType.add)

            def post_layer(bt, gnpt, ABd, fdim):
                """agg_res <- relu(GN(dinv*agg_res + b)), in place."""
                st_ps1 = psacc.tile([GMAX, fdim], dt.float32, tag="stats1")
                st_ps2 = psacc.tile([GMAX, fdim], dt.float32, tag="stats2")
                CH = 3
                assert n_chunks % CH == 0
                btv = bt[:].rearrange("p (c f) -> p c f", c=CH)
                for c3 in range(0, n_chunks, CH):
                    y = agg_res[:, c3:c3 + CH, :]
                    nc.vector.tensor_tensor(
                        out=y, in0=y,
                        in1=dinvt[:, c3:c3 + CH].to_broadcast([P, CH, fdim]),
                        op=mybir.AluOpType.mult)
                    nc.vector.tensor_tensor(
                        out=y, in0=y, in1=btv, op=mybir.AluOpType.add)
                    sq = wpool.tile([P, CH, fdim], dt.float32, tag="sq")
                    nc.vector.tensor_tensor(
                        out=sq[:], in0=y, in1=y, op=mybir.AluOpType.mult)
                    for dc in range(CH):
                        c = c3 + dc
                        nc.tensor.matmul(
                            st_ps1[:], lhsT=St[:, c, :],
                            rhs=agg_res[:, c, :], start=(c == 0),
                            stop=(c == n_chunks - 1))
                        nc.tensor.matmul(
                            st_ps2[:], lhsT=St[:, c, :],
                            rhs=sq[:, dc, :], start=(c == 0),
                            stop=(c == n_chunks - 1))
                # stats -> A, B   (alpha|weight|bias in gnpt)
                stats = wpool.tile([GMAX, 2 * fdim], dt.float32, tag="stf")
                nc.vector.tensor_tensor(
                    out=stats[:, 0:fdim], in0=st_ps1[:],
                    in1=cit[:, 0:1].to_broadcast([GMAX, fdim]),
                    op=mybir.AluOpType.mult)
                nc.vector.tensor_tensor(
                    out=stats[:, fdim:2 * fdim], in0=st_ps2[:],
                    in1=cit[:, 0:1].to_broadcast([GMAX, fdim]),
                    op=mybir.AluOpType.mult)  # [mean | E[x^2]]
                mean = stats[:, 0:fdim]
                ex2 = stats[:, fdim:2 * fdim]
                alpha = gnpt[:, 0:fdim]
                weight = gnpt[:, fdim:2 * fdim]
                bias = gnpt[:, 2 * fdim:3 * fdim]
                am = wpool.tile([GMAX, fdim], dt.float32, tag="am")
                nc.vector.tensor_tensor(out=am[:], in0=alpha, in1=mean,
                                        op=mybir.AluOpType.mult)  # alpha*m
                var = wpool.tile([GMAX, fdim], dt.float32, tag="var")
                # var = E[x^2] - 2*am*m + am^2 = E[x^2] + am*(am - 2m)
                t2 = wpool.tile([GMAX, fdim], dt.float32, tag="t2")
                nc.vector.tensor_scalar(out=t2[:], in0=mean, scalar1=-2.0,
                                        scalar2=None,
                                        op0=mybir.AluOpType.mult)
                nc.vector.tensor_tensor(out=t2[:], in0=t2[:], in1=am[:],
                                        op=mybir.AluOpType.add)
                nc.vector.tensor_tensor(out=t2[:], in0=t2[:], in1=am[:],
                                        op=mybir.AluOpType.mult)
                nc.vector.tensor_tensor(out=var[:], in0=ex2, in1=t2[:],
                                        op=mybir.AluOpType.add)
                istd = wpool.tile([GMAX, fdim], dt.float32, tag="istd")
                nc.vector.tensor_scalar(out=istd[:], in0=var[:], scalar1=EPS,
                                        scalar2=None, op0=mybir.AluOpType.add)
                nc.scalar.activation(istd[:], istd[:],
                                     mybir.ActivationFunctionType.Sqrt)
                nc.vector.reciprocal(istd[:], istd[:])
                AB = wpool.tile([GMAX, 2 * fdim], dt.float32, tag="AB")
                A = AB[:, 0:fdim]
                B = AB[:, fdim:2 * fdim]
                nc.vector.tensor_tensor(out=A, in0=weight, in1=istd[:],
                                        op=mybir.AluOpType.mult)
                nc.vector.tensor_tensor(out=B, in0=A, in1=am[:],
                                        op=mybir.AluOpType.mult)
                nc.vector.tensor_scalar(out=B, in0=B, scalar1=-1.0,
                                        scalar2=None, op0=mybir.AluOpType.mult)
                nc.vector.tensor_tensor(out=B, in0=B, in1=bias,
                                        op=mybir.AluOpType.add)
                nc.sync.dma_start(ABd[:], AB[:])
                # pass 2: y = relu(y*Ae + Be), per <=8-chunk piece (1024-idx
                # dma_gather cap) with gathered per-node A/B
                lo = 0
                while lo < n_chunks:
                    gx = min(8, n_chunks - lo)
                    abt = mpool.tile([P, 8, 2 * fdim], dt.float32, tag="ab",
                                     bufs=2)
                    nc.gpsimd.dma_gather(
                        out_ap=abt[:, 0:gx, :], in_ap=ABd[:],
                        idxs_ap=gb16[:, lo * 8:(lo + gx) * 8],
                        num_idxs=gx * P, num_idxs_reg=gx * P,
                        elem_size=2 * fdim)
                    y = agg_res[:, lo:lo + gx, :]
                    nc.vector.tensor_tensor(
                        out=y, in0=y, in1=abt[:, 0:gx, 0:fdim],
                        op=mybir.AluOpType.mult)
                    nc.vector.tensor_tensor(
                        out=y, in0=y, in1=abt[:, 0:gx, fdim:2 * fdim],
                        op=mybir.AluOpType.add)
                    nc.scalar.activation(y, y,
                                         mybir.ActivationFunctionType.Relu)
                    lo += gx

            # ---------------- layer 1 ----------------
            dense1()
            share(hp_loc1, hp_full1)
            edge_phase(hp_full1)
            post_layer(b1t, gnp1t, AB1d, HID_DIM)

            # ---------------- layer 2 ----------------
            dense2()
            share(hp_loc2, hp_full2)
            edge_phase(hp_full2)
            post_layer(b2t, gnp2t, AB2d, EMB_DIM)

            # ---------------- pooling ----------------
            pl_ps = psacc.tile([GMAX, EMB_DIM], dt.float32, tag="pl")
            for c in range(n_chunks):
                nc.tensor.matmul(pl_ps[:], lhsT=St[:, c, :],
                                 rhs=agg_res[:, c, :],
                                 start=(c == 0), stop=(c == n_chunks - 1))
            plt = wpool.tile([GMAX, EMB_DIM], dt.float32, tag="plt")
            nc.vector.tensor_tensor(
                out=plt[:], in0=pl_ps[:],
                in1=cit[:, 0:1].to_broadcast([GMAX, EMB_DIM]),
                op=mybir.AluOpType.mult)
            nc.sync.dma_start(pool_out[:], plt[:])

    nc.compile()
    return nc


def kernel(x, edge_index, batch, W1, b1, alpha1, weight1, bias1,
           W2, b2, alpha2, weight2, bias2):
    x = np.asarray(x, np.float32)
    edge_index = np.asarray(edge_index, np.int32)
    batch = np.asarray(batch, np.int32)

    src, dst = edge_index[0].astype(np.int64), edge_index[1].astype(np.int64)
    deg = np.bincount(dst, minlength=NUM_NODES).astype(np.float32) + 1.0
    dinv = 1.0 / np.sqrt(deg)

    (node_bounds, graph_bounds, n_shard, n_chunks, remap, NROWS, RQ,
     per_core, cq) = _plan(batch, src, dst)
    C = int(cq.sum())
    qoff = np.concatenate([[0], np.cumsum(cq)]).astype(int)
    qblocks = [n_chunks * int(cq[q]) for q in range(QN)]

    key = (n_shard, n_chunks, tuple(cq))
    if key not in _CACHE:
        _CACHE[key] = _build_program(n_shard, n_chunks, cq)
    nc = _CACHE[key]

    ident = np.eye(P, dtype=np.float32)
    iotw = np.tile(np.arange(P, dtype=np.float32), (P, C))

    in_maps = []
    pool_maps = []
    for k in range(NCORES):
        lo, hi = int(node_bounds[k]), int(node_bounds[k + 1])
        nk = hi - lo
        xk = np.zeros((n_shard, IN_DIM), np.float32)
        xk[:nk] = x[lo:hi]
        xkT = np.ascontiguousarray(xk.T)
        dv = np.zeros(n_shard, np.float32)
        dv[:nk] = dinv[lo:hi]
        dinvk = dv.reshape(n_chunks, P).T.copy()   # [P, n_chunks]

        sl, dl, rq = per_core[k]
        srm = remap[sl]
        # per-quarter zero pad rows (local row of core 2q's first pad node)
        padrow = np.zeros(QN, np.int64)
        for q in range(QN):
            kc = 2 * q  # core whose shard starts quarter q
            nkq = int(node_bounds[kc + 1] - node_bounds[kc])
            padrow[q] = (kc * P + nkq % P) * n_chunks + nkq // P - q * RQ
            assert 0 <= padrow[q] < RQ

        # build dstv (window-major slots) and per-quarter gather streams
        dstv = np.zeros((n_chunks, C, P), np.float32)
        eidx = [np.full(qblocks[q] * P, padrow[q], np.int64) for q in range(QN)]
        order = np.argsort(dl, kind="stable")
        ds_all, ss_all, rq_all = dl[order], srm[order], rq[order]
        win_all = ds_all // P
        for q in range(QN):
            mq = rq_all == q
            dsq, ssq = ds_all[mq], ss_all[mq]
            winq = win_all[mq]
            cnts = np.bincount(winq, minlength=n_chunks)
            offs = np.concatenate([[0], np.cumsum(cnts)])
            for w in range(n_chunks):
                cntw = int(cnts[w])
                if cntw == 0:
                    continue
                sw = ssq[offs[w]:offs[w + 1]] - q * RQ
                dw = (dsq[offs[w]:offs[w + 1]] - w * P).astype(np.float32)
                assert cntw <= cq[q] * P
                # dstv slots: window w, blocks [qoff[q], qoff[q]+cq[q])
                base = int(qoff[q]) * P
                dstv[w].reshape(-1)[base:base + cntw] = dw
                # gather stream slots: quarter q, blocks [w*cq, (w+1)*cq)
                ebase = w * int(cq[q]) * P
                eidx[q][ebase:ebase + cntw] = sw
        dstv1 = dstv.reshape(n_chunks * C, P).T.copy()  # [P, n_chunks*C]
        ei16 = np.concatenate([_wrap16(e) for e in eidx], axis=1)

        glo, ghi = graph_bounds[k], graph_bounds[k + 1]
        ngr = ghi - glo
        assert ngr <= GMAX, ngr
        # S [n_chunks, P, GMAX] one-hot graph membership for local nodes
        gb = np.zeros(n_shard, np.int64)
        gb[:nk] = batch[lo:hi] - glo
        S = np.zeros((n_shard, GMAX), np.float32)
        S[np.arange(nk), gb[:nk]] = 1.0
        S3 = S.reshape(n_chunks, P, GMAX)
        cnts = np.bincount(gb[:nk], minlength=GMAX).astype(np.float32)
        cntinv = (1.0 / np.maximum(cnts, 1.0)).reshape(GMAX, 1).astype(np.float32)
        # AB gather indices: slot (c*128+p) -> graph of node (c, p)
        gbpm = gb.reshape(n_chunks, P)   # [c, p]
        gb16 = _wrap16(gbpm.reshape(-1))

        gnp1 = np.concatenate([
            np.tile(alpha1, (GMAX, 1)), np.tile(weight1, (GMAX, 1)),
            np.tile(bias1, (GMAX, 1))], axis=1).astype(np.float32)
        gnp2 = np.concatenate([
            np.tile(alpha2, (GMAX, 1)), np.tile(weight2, (GMAX, 1)),
            np.tile(bias2, (GMAX, 1))], axis=1).astype(np.float32)

        in_maps.append({
            "xkT": xkT, "W1": np.asarray(W1, np.float32),
            "W2": np.asarray(W2, np.float32),
            "b1r": np.tile(np.asarray(b1, np.float32), (P, 3)),
            "b2r": np.tile(np.asarray(b2, np.float32), (P, 3)),
            "dinvk": np.ascontiguousarray(dinvk), "ident": ident,
            "gnp1": gnp1, "gnp2": gnp2, "cntinv": cntinv,
            "S_in": np.ascontiguousarray(S3),
            "dstv1": np.ascontiguousarray(dstv1),
            "eidx_in": ei16,
            "gb16_in": gb16,
            "iotw_in": iotw,
        })
        pool_maps.append((glo, ghi))

    res = run_bass_kernel_spmd(nc, in_maps, list(range(NCORES)))

    out = np.zeros((NUM_GRAPHS, EMB_DIM), np.float32)
    for k in range(NCORES):
        glo, ghi = pool_maps[k]
        out[glo:ghi] = np.asarray(res.results[k]["pool_out"])[:ghi - glo]
    return out


# revision 18
# speedup vs baseline: 1.0094x; 1.0094x over previous
"""GCN encoder (2x GCNConv + GraphNorm + ReLU + mean-pool) on 8 trn2 cores.

Strategy: graph-aligned node sharding across 8 cores (batch is sorted, so
each core owns a contiguous run of whole graphs -> GraphNorm and pooling are
fully shard-local). Edges are assigned to the core that owns their dst node.
Each core:
  - computes hp = dinv * (x @ W1) for its shard (PE), keeping hp resident in
    SBUF in partition-major chunk layout [128, n_chunks, F],
  - stores hp to DRAM in one contiguous DMA and AllGathers it so every core
    holds the full (partition-major) node table,
  - aggregates messages per 128-dst window: dma_gather streams message rows
    in 1024-row calls (the int16 index limit splits the table into 4
    quarters; a ring of [128, 8, F] tiles per quarter), one DVE is_equal
    builds all one-hot selection matrices of a window at once, C PE matmuls
    accumulate in PSUM, the self-loop is folded in with a DVE add into the
    SBUF-resident agg tile,
  - applies symmetric-norm scaling, bias, GraphNorm (stats via St matmuls,
    per-node A/B coefficients fetched with dma_gather from a [G, 2F] DRAM
    table), ReLU,
  - repeats for layer 2, then computes per-graph mean pooling (PE).
Host side: numpy preprocessing (degrees, sharding, window schedule) + final
assembly of the [128, 64] output.
"""
import numpy as np

import concourse.bass as bass
import concourse.bacc as bacc
import concourse.mybir as mybir
import concourse.tile as tile
from concourse.bass_utils import run_bass_kernel_spmd

dt = mybir.dt

NCORES = 8
NUM_NODES = 100000
NUM_EDGES = 1600000
NUM_GRAPHS = 128
IN_DIM, HID_DIM, EMB_DIM = 128, 64, 64
EPS = 1e-5
GMAX = 32          # max graphs per core (128/8 = 16 avg)
P = 128
QN = 4             # table quarters (int16 gather index range)
CBLK = 8           # blocks per gather call (1024-index ucode cap)

_CACHE = {}


def _shard_plan(batch):
    """Graph-aligned contiguous node shards."""
    counts = np.bincount(batch, minlength=NUM_GRAPHS)
    starts = np.concatenate([[0], np.cumsum(counts)])  # [G+1]
    bounds = [0]
    for k in range(1, NCORES):
        target = k * NUM_NODES // NCORES
        g = int(np.argmin(np.abs(starts - target)))
        bounds.append(int(starts[g]))
    bounds.append(NUM_NODES)
    node_bounds = np.array(bounds)
    graph_bounds = [int(np.searchsorted(starts, b)) for b in bounds]
    return node_bounds, starts, graph_bounds


def _plan(batch, src, dst):
    """Shared host plan: shards, row remap, per-quarter block caps."""
    node_bounds, gstarts, graph_bounds = _shard_plan(batch)
    n_shard = int(np.max(node_bounds[1:] - node_bounds[:-1]))
    # +P guarantees every core has >=1 all-zero pad row (gather target for
    # padding slots in the edge schedule)
    n_shard = ((n_shard + P) // P) * P
    n_chunks = n_shard // P

    # remap node -> partition-major row in hp_full: (core*128 + p)*n_chunks + c
    core_of = np.searchsorted(node_bounds, np.arange(NUM_NODES),
                              side="right") - 1
    local = np.arange(NUM_NODES) - node_bounds[core_of]
    remap = ((core_of * P + local % P) * n_chunks + local // P)

    NROWS = NCORES * P * n_chunks
    RQ = NROWS // QN

    # per-quarter window block caps (global max over cores&windows)
    ecore = core_of[dst]
    per_core = []
    cq = np.zeros(QN, np.int64)
    for k in range(NCORES):
        m = ecore == k
        sl = src[m]
        dl = dst[m] - node_bounds[k]
        rq = remap[sl] // RQ
        for q in range(QN):
            cnt = np.bincount((dl // P)[rq == q], minlength=n_chunks)
            cq[q] = max(cq[q], int(np.max((cnt + P - 1) // P)))
        per_core.append((sl, dl, rq))
    return (node_bounds, graph_bounds, n_shard, n_chunks, remap, NROWS, RQ,
            per_core, cq)


def _wrap16(idx_flat):
    """[N] int -> [128, N/16] int16 in dma_gather wrapped layout
    (index i at [i%16, i//16], replicated x8 over partitions)."""
    n = idx_flat.shape[0]
    assert n % 16 == 0
    w = np.zeros((16, n // 16), np.int16)
    w[np.arange(n) % 16, np.arange(n) // 16] = idx_flat.astype(np.int16)
    return np.tile(w, (8, 1))


def _build_program(n_shard, n_chunks, cq):
    """Bass program; identical across cores. cq = per-quarter block caps."""
    nc = bacc.Bacc("TRN2", target_bir_lowering=False, debug=False,
                   num_devices=NCORES, detect_race_conditions=False,
                   num_swdge_queues=4)

    N_S = n_shard
    cq = [int(v) for v in cq]
    C = sum(cq)                    # blocks per window
    qoff = [0]
    for v in cq:
        qoff.append(qoff[-1] + v)
    NROWS = NCORES * P * n_chunks  # partition-major row table size
    RQ = NROWS // QN
    assert NROWS % QN == 0 and RQ <= 32767
    F = HID_DIM  # 64 (== EMB_DIM)
    # per-quarter gather-call schedule: quarter stream has n_chunks*cq[q]
    # blocks, cut into CBLK-block calls
    qblocks = [n_chunks * cq[q] for q in range(QN)]
    qcalls = [(qb + CBLK - 1) // CBLK for qb in qblocks]
    # wrapped-int16 column base per quarter (8 cols per block)
    qcolbase = [0]
    for qb in qblocks:
        qcolbase.append(qcolbase[-1] + qb * 8)
    EIC = qcolbase[-1]

    # ---------------- inputs ----------------
    xkT = nc.dram_tensor("xkT", [IN_DIM, N_S], dt.float32, kind="ExternalInput")
    W1 = nc.dram_tensor("W1", [IN_DIM, HID_DIM], dt.float32, kind="ExternalInput")
    W2 = nc.dram_tensor("W2", [HID_DIM, EMB_DIM], dt.float32, kind="ExternalInput")
    b1r = nc.dram_tensor("b1r", [P, 3 * HID_DIM], dt.float32, kind="ExternalInput")
    b2r = nc.dram_tensor("b2r", [P, 3 * EMB_DIM], dt.float32, kind="ExternalInput")
    dinvk = nc.dram_tensor("dinvk", [P, n_chunks], dt.float32, kind="ExternalInput")
    ident = nc.dram_tensor("ident", [P, P], dt.float32, kind="ExternalInput")
    gnp1 = nc.dram_tensor("gnp1", [GMAX, 3 * HID_DIM], dt.float32,
                          kind="ExternalInput")  # [alpha|weight|bias]
    gnp2 = nc.dram_tensor("gnp2", [GMAX, 3 * EMB_DIM], dt.float32,
                          kind="ExternalInput")
    cntinv = nc.dram_tensor("cntinv", [GMAX, 1], dt.float32, kind="ExternalInput")
    S_in = nc.dram_tensor("S_in", [n_chunks, P, GMAX], dt.float32,
                          kind="ExternalInput")
    dstv1 = nc.dram_tensor("dstv1", [P, n_chunks * C], dt.float32,
                           kind="ExternalInput")
    eidx_in = nc.dram_tensor("eidx_in", [P, EIC], dt.int16, kind="ExternalInput")
    gb16_in = nc.dram_tensor("gb16_in", [P, n_chunks * 8], dt.int16,
                             kind="ExternalInput")
    iotw_in = nc.dram_tensor("iotw_in", [P, C * P], dt.float32,
                             kind="ExternalInput")

    pool_out = nc.dram_tensor("pool_out", [GMAX, EMB_DIM], dt.float32,
                              kind="ExternalOutput")

    # ---------------- internal DRAM ----------------
    hp_loc1 = nc.dram_tensor("hp_loc1", [P, n_chunks, F], dt.float32)
    hp_loc2 = nc.dram_tensor("hp_loc2", [P, n_chunks, F], dt.float32)
    hp_full1 = nc.dram_tensor("hp_full1", [NROWS, F], dt.float32,
                              addr_space="Shared")
    hp_full2 = nc.dram_tensor("hp_full2", [NROWS, F], dt.float32,
                              addr_space="Shared")
    AB1d = nc.dram_tensor("AB1d", [GMAX, 2 * F], dt.float32)
    AB2d = nc.dram_tensor("AB2d", [GMAX, 2 * F], dt.float32)

    GXD = 9                       # dense-1 chunks per load
    assert n_chunks % GXD == 0

    with tile.TileContext(nc) as tc:
        with (
            tc.tile_pool(name="const", bufs=1) as cpool,
            tc.tile_pool(name="xload", bufs=2) as xpool,
            tc.tile_pool(name="work", bufs=4) as wpool,
            tc.tile_pool(name="msg", bufs=1) as mpool,
            tc.tile_pool(name="psmm", bufs=2, space="PSUM") as psmm,
            tc.tile_pool(name="pstr", bufs=2, space="PSUM") as pstr,
            tc.tile_pool(name="psacc", bufs=1, space="PSUM") as psacc,
        ):
            idt = cpool.tile([P, P], dt.float32)
            nc.sync.dma_start(idt[:], ident[:])
            w1t = cpool.tile([IN_DIM, HID_DIM], dt.float32)
            nc.sync.dma_start(w1t[:], W1[:])
            w2t = cpool.tile([HID_DIM, EMB_DIM], dt.float32)
            nc.sync.dma_start(w2t[:], W2[:])
            dinvt = cpool.tile([P, n_chunks], dt.float32)
            nc.sync.dma_start(dinvt[:], dinvk[:])
            b1t = cpool.tile([P, 3 * HID_DIM], dt.float32)
            nc.sync.dma_start(b1t[:], b1r[:])
            b2t = cpool.tile([P, 3 * EMB_DIM], dt.float32)
            nc.sync.dma_start(b2t[:], b2r[:])
            St = cpool.tile([P, n_chunks, GMAX], dt.float32)
            nc.sync.dma_start(St[:], S_in.ap().rearrange("c p g -> p c g"))
            gnp1t = cpool.tile([GMAX, 3 * HID_DIM], dt.float32)
            nc.sync.dma_start(gnp1t[:], gnp1[:])
            gnp2t = cpool.tile([GMAX, 3 * EMB_DIM], dt.float32)
            nc.sync.dma_start(gnp2t[:], gnp2[:])
            cit = cpool.tile([GMAX, 1], dt.float32)
            nc.sync.dma_start(cit[:], cntinv[:])
            iotw = cpool.tile([P, C * P], dt.float32)
            nc.sync.dma_start(iotw[:], iotw_in[:])
            dvt = cpool.tile([P, n_chunks * C], dt.float32)
            nc.sync.dma_start(dvt[:], dstv1[:])
            ei16 = cpool.tile([P, EIC], dt.int16)
            nc.sync.dma_start(ei16[:], eidx_in[:])
            gb16 = cpool.tile([P, n_chunks * 8], dt.int16)
            nc.sync.dma_start(gb16[:], gb16_in[:])

            # SBUF residents
            hp_res = cpool.tile([P, n_chunks, F], dt.float32)
            agg_res = cpool.tile([P, n_chunks, F], dt.float32)

            def dense1(hp_loc):
                for pc in range(n_chunks // GXD):
                    xt = xpool.tile([IN_DIM, GXD * P], dt.float32, tag="xt")
                    nc.sync.dma_start(xt[:], xkT[:, pc * GXD * P:(pc + 1) * GXD * P])
                    for j in range(GXD):
                        c = pc * GXD + j
                        h_ps = psmm.tile([P, F], dt.float32, tag="mm")
                        nc.tensor.matmul(h_ps[:], lhsT=xt[:, j * P:(j + 1) * P],
                                         rhs=w1t[:], start=True, stop=True)
                        nc.vector.tensor_tensor(
                            out=hp_res[:, c, :], in0=h_ps[:],
                            in1=dinvt[:, c:c + 1].to_broadcast([P, F]),
                            op=mybir.AluOpType.mult)
                    # piecewise store: pipelines behind the remaining dense
                    # compute so the AllGather can start right after the
                    # last piece instead of one monolithic store
                    nc.sync.dma_start(
                        hp_loc[:, pc * GXD:(pc + 1) * GXD, :],
                        hp_res[:, pc * GXD:(pc + 1) * GXD, :])

            def dense2(hp_loc):
                for c in range(n_chunks):
                    tr_ps = pstr.tile([F, P], dt.float32, tag="tr")
                    nc.tensor.transpose(out=tr_ps[:], in_=agg_res[:, c, :],
                                        identity=idt[:])
                    xts = wpool.tile([F, P], dt.float32, tag="xts")
                    nc.scalar.activation(xts[:], tr_ps[:],
                                         mybir.ActivationFunctionType.Copy)
                    h_ps = psmm.tile([P, F], dt.float32, tag="mm")
                    nc.tensor.matmul(h_ps[:], lhsT=xts[:], rhs=w2t[:],
                                     start=True, stop=True)
                    nc.vector.tensor_tensor(
                        out=hp_res[:, c, :], in0=h_ps[:],
                        in1=dinvt[:, c:c + 1].to_broadcast([P, F]),
                        op=mybir.AluOpType.mult)
                    if c % GXD == GXD - 1:
                        pc = c // GXD
                        nc.sync.dma_start(
                            hp_loc[:, pc * GXD:(pc + 1) * GXD, :],
                            hp_res[:, pc * GXD:(pc + 1) * GXD, :])

            def share(hp_loc, hp_full):
                nc.gpsimd.collective_compute(
                    "AllGather", mybir.AluOpType.bypass,
                    replica_groups=[list(range(NCORES))],
                    ins=[hp_loc.ap()], outs=[hp_full.ap()])

            def edge_phase(table_full):
                iotw3 = iotw[:].rearrange("p (c q) -> p c q", q=P)
                qtiles = [{} for _ in range(QN)]   # call idx -> tile
                issued = [0] * QN

                def issue(q):
                    t = issued[q]
                    nb = min(CBLK, qblocks[q] - t * CBLK)
                    mt = mpool.tile([P, CBLK, F], dt.float32,
                                    tag=f"msg{q}", bufs=3)
                    ecol = qcolbase[q] + t * CBLK * 8
                    nc.gpsimd.dma_gather(
                        out_ap=mt[:, 0:nb, :],
                        in_ap=table_full[q * RQ:(q + 1) * RQ, :],
                        idxs_ap=ei16[:, ecol:ecol + nb * 8],
                        num_idxs=nb * P, num_idxs_reg=nb * P,
                        elem_size=F, queue_num=q)
                    qtiles[q][t] = mt
                    issued[q] = t + 1

                for w in range(n_chunks):
                    for q in range(QN):
                        need = (cq[q] * (w + 1) + CBLK - 1) // CBLK
                        while issued[q] < min(need, qcalls[q]):
                            issue(q)
                    S = mpool.tile([P, C, P], dt.float32, tag="sel", bufs=2)
                    nc.vector.tensor_tensor(
                        out=S[:],
                        in0=dvt[:, w * C:(w + 1) * C].to_broadcast([P, C, P]),
                        in1=iotw3, op=mybir.AluOpType.is_equal)
                    ps = psmm.tile([P, F], dt.float32, tag="mm")
                    jj = 0
                    for q in range(QN):
                        for j in range(cq[q]):
                            b = w * cq[q] + j
                            mt = qtiles[q][b // CBLK]
                            nc.tensor.matmul(
                                ps[:], lhsT=S[:, jj, :],
                                rhs=mt[:, b % CBLK, :],
                                start=(jj == 0), stop=(jj == C - 1))
                            jj += 1
                    nc.vector.tensor_tensor(
                        out=agg_res[:, w, :], in0=ps[:],
                        in1=hp_res[:, w, :], op=mybir.AluOpType.add)

            def post_layer(bt, gnpt, ABd, fdim):
                """agg_res <- relu(GN(dinv*agg_res + b)), in place."""
                st_ps1 = psacc.tile([GMAX, fdim], dt.float32, tag="stats1")
                st_ps2 = psacc.tile([GMAX, fdim], dt.float32, tag="stats2")
                CH = 3
                assert n_chunks % CH == 0
                btv = bt[:].rearrange("p (c f) -> p c f", c=CH)
                for c3 in range(0, n_chunks, CH):
                    y = agg_res[:, c3:c3 + CH, :]
                    nc.vector.tensor_tensor(
                        out=y, in0=y,
                        in1=dinvt[:, c3:c3 + CH].to_broadcast([P, CH, fdim]),
                        op=mybir.AluOpType.mult)
                    nc.vector.tensor_tensor(
                        out=y, in0=y, in1=btv, op=mybir.AluOpType.add)
                    sq = wpool.tile([P, CH, fdim], dt.float32, tag="sq")
                    nc.vector.tensor_tensor(
                        out=sq[:], in0=y, in1=y, op=mybir.AluOpType.mult)
                    for dc in range(CH):
                        c = c3 + dc
                        nc.tensor.matmul(
                            st_ps1[:], lhsT=St[:, c, :],
                            rhs=agg_res[:, c, :], start=(c == 0),
                            stop=(c == n_chunks - 1))
                        nc.tensor.matmul(
                            st_ps2[:], lhsT=St[:, c, :],
                            rhs=sq[:, dc, :], start=(c == 0),
                            stop=(c == n_chunks - 1))
                # stats -> A, B   (alpha|weight|bias in gnpt)
                stats = wpool.tile([GMAX, 2 * fdim], dt.float32, tag="stf")
                nc.vector.tensor_tensor(
                    out=stats[:, 0:fdim], in0=st_ps1[:],
                    in1=cit[:, 0:1].to_broadcast([GMAX, fdim]),
                    op=mybir.AluOpType.mult)
                nc.vector.tensor_tensor(
                    out=stats[:, fdim:2 * fdim], in0=st_ps2[:],
                    in1=cit[:, 0:1].to_broadcast([GMAX, fdim]),
                    op=mybir.AluOpType.mult)  # [mean | E[x^2]]
                mean = stats[:, 0:fdim]
                ex2 = stats[:, fdim:2 * fdim]
                alpha = gnpt[:, 0:fdim]
                weight = gnpt[:, fdim:2 * fdim]
                bias = gnpt[:, 2 * fdim:3 * fdim]
                am = wpool.tile([GMAX, fdim], dt.float32, tag="am")
                nc.vector.tensor_tensor(out=am[:], in0=alpha, in1=mean,
                                        op=mybir.AluOpType.mult)  # alpha*m
                var = wpool.tile([GMAX, fdim], dt.float32, tag="var")
                # var = E[x^2] - 2*am*m + am^2 = E[x^2] + am*(am - 2m)
                t2 = wpool.tile([GMAX, fdim], dt.float32, tag="t2")
                nc.vector.tensor_scalar(out=t2[:], in0=mean, scalar1=-2.0,
                                        scalar2=None,
                                        op0=mybir.AluOpType.mult)
                nc.vector.tensor_tensor(out=t2[:], in0=t2[:], in1=am[:],
                                        op=mybir.AluOpType.add)
                nc.vector.tensor_tensor(out=t2[:], in0=t2[:], in1=am[:],
                                        op=mybir.AluOpType.mult)
                nc.vector.tensor_tensor(out=var[:], in0=ex2, in1=t2[:],
                                        op=mybir.AluOpType.add)
                istd = wpool.tile([GMAX, fdim], dt.float32, tag="istd")
                nc.vector.tensor_scalar(out=istd[:], in0=var[:], scalar1=EPS,
                                        scalar2=None, op0=mybir.AluOpType.add)
                nc.scalar.activation(istd[:], istd[:],
                                     mybir.ActivationFunctionType.Sqrt)
                nc.vector.reciprocal(istd[:], istd[:])
                AB = wpool.tile([GMAX, 2 * fdim], dt.float32, tag="AB")
                A = AB[:, 0:fdim]
                B = AB[:, fdim:2 * fdim]
                nc.vector.tensor_tensor(out=A, in0=weight, in1=istd[:],
                                        op=mybir.AluOpType.mult)
                nc.vector.tensor_tensor(out=B, in0=A, in1=am[:],
                                        op=mybir.AluOpType.mult)
                nc.vector.tensor_scalar(out=B, in0=B, scalar1=-1.0,
                                        scalar2=None, op0=mybir.AluOpType.mult)
                nc.vector.tensor_tensor(out=B, in0=B, in1=bias,
                                        op=mybir.AluOpType.add)
                nc.sync.dma_start(ABd[:], AB[:])
                # pass 2: y = relu(y*Ae + Be), per <=8-chunk piece (1024-idx
                # dma_gather cap) with gathered per-node A/B
                lo = 0
                while lo < n_chunks:
                    gx = min(8, n_chunks - lo)
                    abt = mpool.tile([P, 8, 2 * fdim], dt.float32, tag="ab",
                                     bufs=2)
                    nc.gpsimd.dma_gather(
                        out_ap=abt[:, 0:gx, :], in_ap=ABd[:],
                        idxs_ap=gb16[:, lo * 8:(lo + gx) * 8],
                        num_idxs=gx * P, num_idxs_reg=gx * P,
                        elem_size=2 * fdim)
                    y = agg_res[:, lo:lo + gx, :]
                    nc.vector.tensor_tensor(
                        out=y, in0=y, in1=abt[:, 0:gx, 0:fdim],
                        op=mybir.AluOpType.mult)
                    nc.vector.tensor_tensor(
                        out=y, in0=y, in1=abt[:, 0:gx, fdim:2 * fdim],
                        op=mybir.AluOpType.add)
                    nc.scalar.activation(y, y,
                                         mybir.ActivationFunctionType.Relu)
                    lo += gx

            # ---------------- layer 1 ----------------
            dense1(hp_loc1)
            share(hp_loc1, hp_full1)
            edge_phase(hp_full1)
            post_layer(b1t, gnp1t, AB1d, HID_DIM)

            # ---------------- layer 2 ----------------
            dense2(hp_loc2)
            share(hp_loc2, hp_full2)
            edge_phase(hp_full2)
            post_layer(b2t, gnp2t, AB2d, EMB_DIM)

            # ---------------- pooling ----------------
            pl_ps = psacc.tile([GMAX, EMB_DIM], dt.float32, tag="pl")
            for c in range(n_chunks):
                nc.tensor.matmul(pl_ps[:], lhsT=St[:, c, :],
                                 rhs=agg_res[:, c, :],
                                 start=(c == 0), stop=(c == n_chunks - 1))
            plt = wpool.tile([GMAX, EMB_DIM], dt.float32, tag="plt")
            nc.vector.tensor_tensor(
                out=plt[:], in0=pl_ps[:],
                in1=cit[:, 0:1].to_broadcast([GMAX, EMB_DIM]),
                op=mybir.AluOpType.mult)
            nc.sync.dma_start(pool_out[:], plt[:])

    nc.compile()
    return nc


def kernel(x, edge_index, batch, W1, b1, alpha1, weight1, bias1,
           W2, b2, alpha2, weight2, bias2):
    x = np.asarray(x, np.float32)
    edge_index = np.asarray(edge_index, np.int32)
    batch = np.asarray(batch, np.int32)

    src, dst = edge_index[0].astype(np.int64), edge_index[1].astype(np.int64)
    deg = np.bincount(dst, minlength=NUM_NODES).astype(np.float32) + 1.0
    dinv = 1.0 / np.sqrt(deg)

    (node_bounds, graph_bounds, n_shard, n_chunks, remap, NROWS, RQ,
     per_core, cq) = _plan(batch, src, dst)
    C = int(cq.sum())
    qoff = np.concatenate([[0], np.cumsum(cq)]).astype(int)
    qblocks = [n_chunks * int(cq[q]) for q in range(QN)]

    key = (n_shard, n_chunks, tuple(cq))
    if key not in _CACHE:
        _CACHE[key] = _build_program(n_shard, n_chunks, cq)
    nc = _CACHE[key]

    ident = np.eye(P, dtype=np.float32)
    iotw = np.tile(np.arange(P, dtype=np.float32), (P, C))

    in_maps = []
    pool_maps = []
    for k in range(NCORES):
        lo, hi = int(node_bounds[k]), int(node_bounds[k + 1])
        nk = hi - lo
        xk = np.zeros((n_shard, IN_DIM), np.float32)
        xk[:nk] = x[lo:hi]
        xkT = np.ascontiguousarray(xk.T)
        dv = np.zeros(n_shard, np.float32)
        dv[:nk] = dinv[lo:hi]
        dinvk = dv.reshape(n_chunks, P).T.copy()   # [P, n_chunks]

        sl, dl, rq = per_core[k]
        srm = remap[sl]
        # per-quarter zero pad rows (local row of core 2q's first pad node)
        padrow = np.zeros(QN, np.int64)
        for q in range(QN):
            kc = 2 * q  # core whose shard starts quarter q
            nkq = int(node_bounds[kc + 1] - node_bounds[kc])
            padrow[q] = (kc * P + nkq % P) * n_chunks + nkq // P - q * RQ
            assert 0 <= padrow[q] < RQ

        # build dstv (window-major slots) and per-quarter gather streams
        dstv = np.zeros((n_chunks, C, P), np.float32)
        eidx = [np.full(qblocks[q] * P, padrow[q], np.int64) for q in range(QN)]
        order = np.argsort(dl, kind="stable")
        ds_all, ss_all, rq_all = dl[order], srm[order], rq[order]
        win_all = ds_all // P
        for q in range(QN):
            mq = rq_all == q
            dsq, ssq = ds_all[mq], ss_all[mq]
            winq = win_all[mq]
            cnts = np.bincount(winq, minlength=n_chunks)
            offs = np.concatenate([[0], np.cumsum(cnts)])
            for w in range(n_chunks):
                cntw = int(cnts[w])
                if cntw == 0:
                    continue
                sw = ssq[offs[w]:offs[w + 1]] - q * RQ
                dw = (dsq[offs[w]:offs[w + 1]] - w * P).astype(np.float32)
                assert cntw <= cq[q] * P
                # dstv slots: window w, blocks [qoff[q], qoff[q]+cq[q])
                base = int(qoff[q]) * P
                dstv[w].reshape(-1)[base:base + cntw] = dw
                # gather stream slots: quarter q, blocks [w*cq, (w+1)*cq)
                ebase = w * int(cq[q]) * P
                eidx[q][ebase:ebase + cntw] = sw
        dstv1 = dstv.reshape(n_chunks * C, P).T.copy()  # [P, n_chunks*C]
        ei16 = np.concatenate([_wrap16(e) for e in eidx], axis=1)

        glo, ghi = graph_bounds[k], graph_bounds[k + 1]
        ngr = ghi - glo
        assert ngr <= GMAX, ngr
        # S [n_chunks, P, GMAX] one-hot graph membership for local nodes
        gb = np.zeros(n_shard, np.int64)
        gb[:nk] = batch[lo:hi] - glo
        S = np.zeros((n_shard, GMAX), np.float32)
        S[np.arange(nk), gb[:nk]] = 1.0
        S3 = S.reshape(n_chunks, P, GMAX)
        cnts = np.bincount(gb[:nk], minlength=GMAX).astype(np.float32)
        cntinv = (1.0 / np.maximum(cnts, 1.0)).reshape(GMAX, 1).astype(np.float32)
        # AB gather indices: slot (c*128+p) -> graph of node (c, p)
        gbpm = gb.reshape(n_chunks, P)   # [c, p]
        gb16 = _wrap16(gbpm.reshape(-1))

        gnp1 = np.concatenate([
            np.tile(alpha1, (GMAX, 1)), np.tile(weight1, (GMAX, 1)),
            np.tile(bias1, (GMAX, 1))], axis=1).astype(np.float32)
        gnp2 = np.concatenate([
            np.tile(alpha2, (GMAX, 1)), np.tile(weight2, (GMAX, 1)),
            np.tile(bias2, (GMAX, 1))], axis=1).astype(np.float32)

        in_maps.append({
            "xkT": xkT, "W1": np.asarray(W1, np.float32),
            "W2": np.asarray(W2, np.float32),
            "b1r": np.tile(np.asarray(b1, np.float32), (P, 3)),
            "b2r": np.tile(np.asarray(b2, np.float32), (P, 3)),
            "dinvk": np.ascontiguousarray(dinvk), "ident": ident,
            "gnp1": gnp1, "gnp2": gnp2, "cntinv": cntinv,
            "S_in": np.ascontiguousarray(S3),
            "dstv1": np.ascontiguousarray(dstv1),
            "eidx_in": ei16,
            "gb16_in": gb16,
            "iotw_in": iotw,
        })
        pool_maps.append((glo, ghi))

    res = run_bass_kernel_spmd(nc, in_maps, list(range(NCORES)))

    out = np.zeros((NUM_GRAPHS, EMB_DIM), np.float32)
    for k in range(NCORES):
        glo, ghi = pool_maps[k]
        out[glo:ghi] = np.asarray(res.results[k]["pool_out"])[:ghi - glo]
    return out


# revision 23
# speedup vs baseline: 1.0141x; 1.0047x over previous
"""GCN encoder (2x GCNConv + GraphNorm + ReLU + mean-pool) on 8 trn2 cores.

Strategy: graph-aligned node sharding across 8 cores (batch is sorted, so
each core owns a contiguous run of whole graphs -> GraphNorm and pooling are
fully shard-local). Edges are assigned to the core that owns their dst node.
Each core:
  - computes hp = dinv * (x @ W1) for its shard (PE), keeping hp resident in
    SBUF in partition-major chunk layout [128, n_chunks, F],
  - stores hp to DRAM in one contiguous DMA and AllGathers it so every core
    holds the full (partition-major) node table,
  - aggregates messages per 128-dst window: dma_gather streams message rows
    in 1024-row calls (the int16 index limit splits the table into 4
    quarters; a ring of [128, 8, F] tiles per quarter), one DVE is_equal
    builds all one-hot selection matrices of a window at once, C PE matmuls
    accumulate in PSUM, the self-loop is folded in with a DVE add into the
    SBUF-resident agg tile,
  - applies symmetric-norm scaling, bias, GraphNorm (stats via St matmuls,
    per-node A/B coefficients fetched with dma_gather from a [G, 2F] DRAM
    table), ReLU,
  - repeats for layer 2, then computes per-graph mean pooling (PE).
Host side: numpy preprocessing (degrees, sharding, window schedule) + final
assembly of the [128, 64] output.
"""
import numpy as np

import concourse.bass as bass
import concourse.bacc as bacc
import concourse.mybir as mybir
import concourse.tile as tile
from concourse.bass_utils import run_bass_kernel_spmd

dt = mybir.dt

NCORES = 8
NUM_NODES = 100000
NUM_EDGES = 1600000
NUM_GRAPHS = 128
IN_DIM, HID_DIM, EMB_DIM = 128, 64, 64
EPS = 1e-5
GMAX = 32          # max graphs per core (128/8 = 16 avg)
P = 128
QN = 4             # table quarters (int16 gather index range)
CBLK = 8           # blocks per gather call (1024-index ucode cap)

_CACHE = {}


def _shard_plan(batch):
    """Graph-aligned contiguous node shards."""
    counts = np.bincount(batch, minlength=NUM_GRAPHS)
    starts = np.concatenate([[0], np.cumsum(counts)])  # [G+1]
    bounds = [0]
    for k in range(1, NCORES):
        target = k * NUM_NODES // NCORES
        g = int(np.argmin(np.abs(starts - target)))
        bounds.append(int(starts[g]))
    bounds.append(NUM_NODES)
    node_bounds = np.array(bounds)
    graph_bounds = [int(np.searchsorted(starts, b)) for b in bounds]
    return node_bounds, starts, graph_bounds


def _plan(batch, src, dst):
    """Shared host plan: shards, row remap, per-quarter block caps."""
    node_bounds, gstarts, graph_bounds = _shard_plan(batch)
    n_shard = int(np.max(node_bounds[1:] - node_bounds[:-1]))
    # +P guarantees every core has >=1 all-zero pad row (gather target for
    # padding slots in the edge schedule)
    n_shard = ((n_shard + P) // P) * P
    n_chunks = n_shard // P

    # remap node -> partition-major row in hp_full: (core*128 + p)*n_chunks + c
    core_of = np.searchsorted(node_bounds, np.arange(NUM_NODES),
                              side="right") - 1
    local = np.arange(NUM_NODES) - node_bounds[core_of]
    remap = ((core_of * P + local % P) * n_chunks + local // P)

    NROWS = NCORES * P * n_chunks
    RQ = NROWS // QN

    # per-quarter window block caps (global max over cores&windows)
    ecore = core_of[dst]
    per_core = []
    cq = np.zeros(QN, np.int64)
    for k in range(NCORES):
        m = ecore == k
        sl = src[m]
        dl = dst[m] - node_bounds[k]
        rq = remap[sl] // RQ
        for q in range(QN):
            cnt = np.bincount((dl // P)[rq == q], minlength=n_chunks)
            cq[q] = max(cq[q], int(np.max((cnt + P - 1) // P)))
        per_core.append((sl, dl, rq))
    return (node_bounds, graph_bounds, n_shard, n_chunks, remap, NROWS, RQ,
            per_core, cq)


def _wrap16(idx_flat):
    """[N] int -> [128, N/16] int16 in dma_gather wrapped layout
    (index i at [i%16, i//16], replicated x8 over partitions)."""
    n = idx_flat.shape[0]
    assert n % 16 == 0
    w = np.zeros((16, n // 16), np.int16)
    w[np.arange(n) % 16, np.arange(n) // 16] = idx_flat.astype(np.int16)
    return np.tile(w, (8, 1))


def _build_program(n_shard, n_chunks, cq):
    """Bass program; identical across cores. cq = per-quarter block caps."""
    nc = bacc.Bacc("TRN2", target_bir_lowering=False, debug=False,
                   num_devices=NCORES, detect_race_conditions=False,
                   num_swdge_queues=4)

    N_S = n_shard
    cq = [int(v) for v in cq]
    C = sum(cq)                    # blocks per window
    qoff = [0]
    for v in cq:
        qoff.append(qoff[-1] + v)
    NROWS = NCORES * P * n_chunks  # partition-major row table size
    RQ = NROWS // QN
    assert NROWS % QN == 0 and RQ <= 32767
    F = HID_DIM  # 64 (== EMB_DIM)
    # per-quarter gather-call schedule: quarter stream has n_chunks*cq[q]
    # blocks, cut into CBLK-block calls
    qblocks = [n_chunks * cq[q] for q in range(QN)]
    qcalls = [(qb + CBLK - 1) // CBLK for qb in qblocks]
    # wrapped-int16 column base per quarter (8 cols per block)
    qcolbase = [0]
    for qb in qblocks:
        qcolbase.append(qcolbase[-1] + qb * 8)
    EIC = qcolbase[-1]

    # ---------------- inputs ----------------
    xkT = nc.dram_tensor("xkT", [IN_DIM, N_S], dt.float32, kind="ExternalInput")
    W1 = nc.dram_tensor("W1", [IN_DIM, HID_DIM], dt.float32, kind="ExternalInput")
    W2 = nc.dram_tensor("W2", [HID_DIM, EMB_DIM], dt.float32, kind="ExternalInput")
    b1r = nc.dram_tensor("b1r", [P, 11 * HID_DIM], dt.float32, kind="ExternalInput")
    b2r = nc.dram_tensor("b2r", [P, 11 * EMB_DIM], dt.float32, kind="ExternalInput")
    dinvk = nc.dram_tensor("dinvk", [P, n_chunks], dt.float32, kind="ExternalInput")
    ident = nc.dram_tensor("ident", [P, P], dt.float32, kind="ExternalInput")
    gnp1 = nc.dram_tensor("gnp1", [GMAX, 3 * HID_DIM], dt.float32,
                          kind="ExternalInput")  # [alpha|weight|bias]
    gnp2 = nc.dram_tensor("gnp2", [GMAX, 3 * EMB_DIM], dt.float32,
                          kind="ExternalInput")
    cntinv = nc.dram_tensor("cntinv", [GMAX, 1], dt.float32, kind="ExternalInput")
    S_in = nc.dram_tensor("S_in", [n_chunks, P, GMAX], dt.float32,
                          kind="ExternalInput")
    dstv1 = nc.dram_tensor("dstv1", [P, n_chunks * C], dt.float32,
                           kind="ExternalInput")
    eidx_in = nc.dram_tensor("eidx_in", [P, EIC], dt.int16, kind="ExternalInput")
    gb16_in = nc.dram_tensor("gb16_in", [P, n_chunks * 8], dt.int16,
                             kind="ExternalInput")
    iotw_in = nc.dram_tensor("iotw_in", [P, C * P], dt.float32,
                             kind="ExternalInput")

    pool_out = nc.dram_tensor("pool_out", [GMAX, EMB_DIM], dt.float32,
                              kind="ExternalOutput")

    # ---------------- internal DRAM ----------------
    hp_loc1 = nc.dram_tensor("hp_loc1", [P, n_chunks, F], dt.float32)
    hp_loc2 = nc.dram_tensor("hp_loc2", [P, n_chunks, F], dt.float32)
    hp_full1 = nc.dram_tensor("hp_full1", [NROWS, F], dt.float32,
                              addr_space="Shared")
    hp_full2 = nc.dram_tensor("hp_full2", [NROWS, F], dt.float32,
                              addr_space="Shared")
    AB1d = nc.dram_tensor("AB1d", [GMAX, 2 * F], dt.float32)
    AB2d = nc.dram_tensor("AB2d", [GMAX, 2 * F], dt.float32)

    GXD = 11                      # dense-1 chunks per load
    assert n_chunks % GXD == 0

    with tile.TileContext(nc) as tc:
        with (
            tc.tile_pool(name="const", bufs=1) as cpool,
            tc.tile_pool(name="xload", bufs=2) as xpool,
            tc.tile_pool(name="work", bufs=4) as wpool,
            tc.tile_pool(name="msg", bufs=1) as mpool,
            tc.tile_pool(name="psmm", bufs=2, space="PSUM") as psmm,
            tc.tile_pool(name="pstr", bufs=2, space="PSUM") as pstr,
            tc.tile_pool(name="psacc", bufs=1, space="PSUM") as psacc,
        ):
            idt = cpool.tile([P, P], dt.float32)
            nc.sync.dma_start(idt[:], ident[:])
            w1t = cpool.tile([IN_DIM, HID_DIM], dt.float32)
            nc.sync.dma_start(w1t[:], W1[:])
            w2t = cpool.tile([HID_DIM, EMB_DIM], dt.float32)
            nc.sync.dma_start(w2t[:], W2[:])
            dinvt = cpool.tile([P, n_chunks], dt.float32)
            nc.sync.dma_start(dinvt[:], dinvk[:])
            b1t = cpool.tile([P, 11 * HID_DIM], dt.float32)
            nc.sync.dma_start(b1t[:], b1r[:])
            b2t = cpool.tile([P, 11 * EMB_DIM], dt.float32)
            nc.sync.dma_start(b2t[:], b2r[:])
            St = cpool.tile([P, n_chunks, GMAX], dt.float32)
            nc.sync.dma_start(St[:], S_in.ap().rearrange("c p g -> p c g"))
            gnp1t = cpool.tile([GMAX, 3 * HID_DIM], dt.float32)
            nc.sync.dma_start(gnp1t[:], gnp1[:])
            gnp2t = cpool.tile([GMAX, 3 * EMB_DIM], dt.float32)
            nc.sync.dma_start(gnp2t[:], gnp2[:])
            cit = cpool.tile([GMAX, 1], dt.float32)
            nc.sync.dma_start(cit[:], cntinv[:])
            iotw = cpool.tile([P, C * P], dt.float32)
            nc.sync.dma_start(iotw[:], iotw_in[:])
            dvt = cpool.tile([P, n_chunks * C], dt.float32)
            nc.sync.dma_start(dvt[:], dstv1[:])
            ei16 = cpool.tile([P, EIC], dt.int16)
            nc.sync.dma_start(ei16[:], eidx_in[:])
            gb16 = cpool.tile([P, n_chunks * 8], dt.int16)
            nc.sync.dma_start(gb16[:], gb16_in[:])

            # SBUF residents
            hp_res = cpool.tile([P, n_chunks, F], dt.float32)
            agg_res = cpool.tile([P, n_chunks, F], dt.float32)

            def dense1(hp_loc):
                for pc in range(n_chunks // GXD):
                    xt = xpool.tile([IN_DIM, GXD * P], dt.float32, tag="xt")
                    nc.sync.dma_start(xt[:], xkT[:, pc * GXD * P:(pc + 1) * GXD * P])
                    for j in range(GXD):
                        c = pc * GXD + j
                        h_ps = psmm.tile([P, F], dt.float32, tag="mm")
                        nc.tensor.matmul(h_ps[:], lhsT=xt[:, j * P:(j + 1) * P],
                                         rhs=w1t[:], start=True, stop=True)
                        nc.vector.tensor_tensor(
                            out=hp_res[:, c, :], in0=h_ps[:],
                            in1=dinvt[:, c:c + 1].to_broadcast([P, F]),
                            op=mybir.AluOpType.mult)
                    # piecewise store: pipelines behind the remaining dense
                    # compute so the AllGather can start right after the
                    # last piece instead of one monolithic store
                    nc.sync.dma_start(
                        hp_loc[:, pc * GXD:(pc + 1) * GXD, :],
                        hp_res[:, pc * GXD:(pc + 1) * GXD, :])

            def dense2(hp_loc):
                for c in range(n_chunks):
                    tr_ps = pstr.tile([F, P], dt.float32, tag="tr")
                    nc.tensor.transpose(out=tr_ps[:], in_=agg_res[:, c, :],
                                        identity=idt[:])
                    xts = wpool.tile([F, P], dt.float32, tag="xts")
                    nc.scalar.activation(xts[:], tr_ps[:],
                                         mybir.ActivationFunctionType.Copy)
                    h_ps = psmm.tile([P, F], dt.float32, tag="mm")
                    nc.tensor.matmul(h_ps[:], lhsT=xts[:], rhs=w2t[:],
                                     start=True, stop=True)
                    nc.vector.tensor_tensor(
                        out=hp_res[:, c, :], in0=h_ps[:],
                        in1=dinvt[:, c:c + 1].to_broadcast([P, F]),
                        op=mybir.AluOpType.mult)
                    if c % GXD == GXD - 1:
                        pc = c // GXD
                        nc.sync.dma_start(
                            hp_loc[:, pc * GXD:(pc + 1) * GXD, :],
                            hp_res[:, pc * GXD:(pc + 1) * GXD, :])

            def share(hp_loc, hp_full):
                nc.gpsimd.collective_compute(
                    "AllGather", mybir.AluOpType.bypass,
                    replica_groups=[list(range(NCORES))],
                    ins=[hp_loc.ap()], outs=[hp_full.ap()])

            def edge_phase(table_full):
                iotw3 = iotw[:].rearrange("p (c q) -> p c q", q=P)
                qtiles = [{} for _ in range(QN)]   # call idx -> tile
                issued = [0] * QN

                def issue(q):
                    t = issued[q]
                    nb = min(CBLK, qblocks[q] - t * CBLK)
                    mt = mpool.tile([P, CBLK, F], dt.float32,
                                    tag=f"msg{q}", bufs=3)
                    ecol = qcolbase[q] + t * CBLK * 8
                    nc.gpsimd.dma_gather(
                        out_ap=mt[:, 0:nb, :],
                        in_ap=table_full[q * RQ:(q + 1) * RQ, :],
                        idxs_ap=ei16[:, ecol:ecol + nb * 8],
                        num_idxs=nb * P, num_idxs_reg=nb * P,
                        elem_size=F, queue_num=q)
                    qtiles[q][t] = mt
                    issued[q] = t + 1

                for w in range(n_chunks):
                    for q in range(QN):
                        need = (cq[q] * (w + 1) + CBLK - 1) // CBLK
                        while issued[q] < min(need, qcalls[q]):
                            issue(q)
                    S = mpool.tile([P, C, P], dt.float32, tag="sel", bufs=2)
                    nc.vector.tensor_tensor(
                        out=S[:],
                        in0=dvt[:, w * C:(w + 1) * C].to_broadcast([P, C, P]),
                        in1=iotw3, op=mybir.AluOpType.is_equal)
                    ps = psmm.tile([P, F], dt.float32, tag="mm")
                    jj = 0
                    for q in range(QN):
                        for j in range(cq[q]):
                            b = w * cq[q] + j
                            mt = qtiles[q][b // CBLK]
                            nc.tensor.matmul(
                                ps[:], lhsT=S[:, jj, :],
                                rhs=mt[:, b % CBLK, :],
                                start=(jj == 0), stop=(jj == C - 1))
                            jj += 1
                    nc.vector.tensor_tensor(
                        out=agg_res[:, w, :], in0=ps[:],
                        in1=hp_res[:, w, :], op=mybir.AluOpType.add)

            def post_layer(bt, gnpt, ABd, fdim):
                """agg_res <- relu(GN(dinv*agg_res + b)), in place."""
                st_ps1 = psacc.tile([GMAX, fdim], dt.float32, tag="stats1")
                st_ps2 = psacc.tile([GMAX, fdim], dt.float32, tag="stats2")
                CH = 11
                assert n_chunks % CH == 0
                btv = bt[:].rearrange("p (c f) -> p c f", c=CH)
                for c3 in range(0, n_chunks, CH):
                    y = agg_res[:, c3:c3 + CH, :]
                    nc.vector.tensor_tensor(
                        out=y, in0=y,
                        in1=dinvt[:, c3:c3 + CH].to_broadcast([P, CH, fdim]),
                        op=mybir.AluOpType.mult)
                    nc.vector.tensor_tensor(
                        out=y, in0=y, in1=btv, op=mybir.AluOpType.add)
                    sq = wpool.tile([P, CH, fdim], dt.float32, tag="sq")
                    nc.vector.tensor_tensor(
                        out=sq[:], in0=y, in1=y, op=mybir.AluOpType.mult)
                    for dc in range(CH):
                        c = c3 + dc
                        nc.tensor.matmul(
                            st_ps1[:], lhsT=St[:, c, :],
                            rhs=agg_res[:, c, :], start=(c == 0),
                            stop=(c == n_chunks - 1))
                        nc.tensor.matmul(
                            st_ps2[:], lhsT=St[:, c, :],
                            rhs=sq[:, dc, :], start=(c == 0),
                            stop=(c == n_chunks - 1))
                # stats -> A, B   (alpha|weight|bias in gnpt)
                stats = wpool.tile([GMAX, 2 * fdim], dt.float32, tag="stf")
                nc.vector.tensor_tensor(
                    out=stats[:, 0:fdim], in0=st_ps1[:],
                    in1=cit[:, 0:1].to_broadcast([GMAX, fdim]),
                    op=mybir.AluOpType.mult)
                nc.vector.tensor_tensor(
                    out=stats[:, fdim:2 * fdim], in0=st_ps2[:],
                    in1=cit[:, 0:1].to_broadcast([GMAX, fdim]),
                    op=mybir.AluOpType.mult)  # [mean | E[x^2]]
                mean = stats[:, 0:fdim]
                ex2 = stats[:, fdim:2 * fdim]
                alpha = gnpt[:, 0:fdim]
                weight = gnpt[:, fdim:2 * fdim]
                bias = gnpt[:, 2 * fdim:3 * fdim]
                am = wpool.tile([GMAX, fdim], dt.float32, tag="am")
                nc.vector.tensor_tensor(out=am[:], in0=alpha, in1=mean,
                                        op=mybir.AluOpType.mult)  # alpha*m
                var = wpool.tile([GMAX, fdim], dt.float32, tag="var")
                # var = E[x^2] - 2*am*m + am^2 = E[x^2] + am*(am - 2m)
                t2 = wpool.tile([GMAX, fdim], dt.float32, tag="t2")
                nc.vector.tensor_scalar(out=t2[:], in0=mean, scalar1=-2.0,
                                        scalar2=None,
                                        op0=mybir.AluOpType.mult)
                nc.vector.tensor_tensor(out=t2[:], in0=t2[:], in1=am[:],
                                        op=mybir.AluOpType.add)
                nc.vector.tensor_tensor(out=t2[:], in0=t2[:], in1=am[:],
                                        op=mybir.AluOpType.mult)
                nc.vector.tensor_tensor(out=var[:], in0=ex2, in1=t2[:],
                                        op=mybir.AluOpType.add)
                istd = wpool.tile([GMAX, fdim], dt.float32, tag="istd")
                nc.vector.tensor_scalar(out=istd[:], in0=var[:], scalar1=EPS,
                                        scalar2=None, op0=mybir.AluOpType.add)
                nc.scalar.activation(istd[:], istd[:],
                                     mybir.ActivationFunctionType.Sqrt)
                nc.vector.reciprocal(istd[:], istd[:])
                AB = wpool.tile([GMAX, 2 * fdim], dt.float32, tag="AB")
                A = AB[:, 0:fdim]
                B = AB[:, fdim:2 * fdim]
                nc.vector.tensor_tensor(out=A, in0=weight, in1=istd[:],
                                        op=mybir.AluOpType.mult)
                nc.vector.tensor_tensor(out=B, in0=A, in1=am[:],
                                        op=mybir.AluOpType.mult)
                nc.vector.tensor_scalar(out=B, in0=B, scalar1=-1.0,
                                        scalar2=None, op0=mybir.AluOpType.mult)
                nc.vector.tensor_tensor(out=B, in0=B, in1=bias,
                                        op=mybir.AluOpType.add)
                nc.sync.dma_start(ABd[:], AB[:])
                # pass 2: y = relu(y*Ae + Be), per <=8-chunk piece (1024-idx
                # dma_gather cap) with gathered per-node A/B
                lo = 0
                while lo < n_chunks:
                    gx = min(8, n_chunks - lo)
                    abt = mpool.tile([P, 8, 2 * fdim], dt.float32, tag="ab",
                                     bufs=2)
                    nc.gpsimd.dma_gather(
                        out_ap=abt[:, 0:gx, :], in_ap=ABd[:],
                        idxs_ap=gb16[:, lo * 8:(lo + gx) * 8],
                        num_idxs=gx * P, num_idxs_reg=gx * P,
                        elem_size=2 * fdim)
                    y = agg_res[:, lo:lo + gx, :]
                    nc.vector.tensor_tensor(
                        out=y, in0=y, in1=abt[:, 0:gx, 0:fdim],
                        op=mybir.AluOpType.mult)
                    nc.vector.tensor_tensor(
                        out=y, in0=y, in1=abt[:, 0:gx, fdim:2 * fdim],
                        op=mybir.AluOpType.add)
                    nc.scalar.activation(y, y,
                                         mybir.ActivationFunctionType.Relu)
                    lo += gx

            # ---------------- layer 1 ----------------
            dense1(hp_loc1)
            share(hp_loc1, hp_full1)
            edge_phase(hp_full1)
            post_layer(b1t, gnp1t, AB1d, HID_DIM)

            # ---------------- layer 2 ----------------
            dense2(hp_loc2)
            share(hp_loc2, hp_full2)
            edge_phase(hp_full2)
            post_layer(b2t, gnp2t, AB2d, EMB_DIM)

            # ---------------- pooling ----------------
            pl_ps = psacc.tile([GMAX, EMB_DIM], dt.float32, tag="pl")
            for c in range(n_chunks):
                nc.tensor.matmul(pl_ps[:], lhsT=St[:, c, :],
                                 rhs=agg_res[:, c, :],
                                 start=(c == 0), stop=(c == n_chunks - 1))
            plt = wpool.tile([GMAX, EMB_DIM], dt.float32, tag="plt")
            nc.vector.tensor_tensor(
                out=plt[:], in0=pl_ps[:],
                in1=cit[:, 0:1].to_broadcast([GMAX, EMB_DIM]),
                op=mybir.AluOpType.mult)
            nc.sync.dma_start(pool_out[:], plt[:])

    nc.compile()
    return nc


def kernel(x, edge_index, batch, W1, b1, alpha1, weight1, bias1,
           W2, b2, alpha2, weight2, bias2):
    x = np.asarray(x, np.float32)
    edge_index = np.asarray(edge_index, np.int32)
    batch = np.asarray(batch, np.int32)

    src, dst = edge_index[0].astype(np.int64), edge_index[1].astype(np.int64)
    deg = np.bincount(dst, minlength=NUM_NODES).astype(np.float32) + 1.0
    dinv = 1.0 / np.sqrt(deg)

    (node_bounds, graph_bounds, n_shard, n_chunks, remap, NROWS, RQ,
     per_core, cq) = _plan(batch, src, dst)
    C = int(cq.sum())
    qoff = np.concatenate([[0], np.cumsum(cq)]).astype(int)
    qblocks = [n_chunks * int(cq[q]) for q in range(QN)]

    key = (n_shard, n_chunks, tuple(cq))
    if key not in _CACHE:
        _CACHE[key] = _build_program(n_shard, n_chunks, cq)
    nc = _CACHE[key]

    ident = np.eye(P, dtype=np.float32)
    iotw = np.tile(np.arange(P, dtype=np.float32), (P, C))

    in_maps = []
    pool_maps = []
    for k in range(NCORES):
        lo, hi = int(node_bounds[k]), int(node_bounds[k + 1])
        nk = hi - lo
        xk = np.zeros((n_shard, IN_DIM), np.float32)
        xk[:nk] = x[lo:hi]
        xkT = np.ascontiguousarray(xk.T)
        dv = np.zeros(n_shard, np.float32)
        dv[:nk] = dinv[lo:hi]
        dinvk = dv.reshape(n_chunks, P).T.copy()   # [P, n_chunks]

        sl, dl, rq = per_core[k]
        srm = remap[sl]
        # per-quarter zero pad rows (local row of core 2q's first pad node)
        padrow = np.zeros(QN, np.int64)
        for q in range(QN):
            kc = 2 * q  # core whose shard starts quarter q
            nkq = int(node_bounds[kc + 1] - node_bounds[kc])
            padrow[q] = (kc * P + nkq % P) * n_chunks + nkq // P - q * RQ
            assert 0 <= padrow[q] < RQ

        # build dstv (window-major slots) and per-quarter gather streams
        dstv = np.zeros((n_chunks, C, P), np.float32)
        eidx = [np.full(qblocks[q] * P, padrow[q], np.int64) for q in range(QN)]
        order = np.argsort(dl, kind="stable")
        ds_all, ss_all, rq_all = dl[order], srm[order], rq[order]
        win_all = ds_all // P
        for q in range(QN):
            mq = rq_all == q
            dsq, ssq = ds_all[mq], ss_all[mq]
            winq = win_all[mq]
            cnts = np.bincount(winq, minlength=n_chunks)
            offs = np.concatenate([[0], np.cumsum(cnts)])
            for w in range(n_chunks):
                cntw = int(cnts[w])
                if cntw == 0:
                    continue
                sw = ssq[offs[w]:offs[w + 1]] - q * RQ
                dw = (dsq[offs[w]:offs[w + 1]] - w * P).astype(np.float32)
                assert cntw <= cq[q] * P
                # dstv slots: window w, blocks [qoff[q], qoff[q]+cq[q])
                base = int(qoff[q]) * P
                dstv[w].reshape(-1)[base:base + cntw] = dw
                # gather stream slots: quarter q, blocks [w*cq, (w+1)*cq)
                ebase = w * int(cq[q]) * P
                eidx[q][ebase:ebase + cntw] = sw
        dstv1 = dstv.reshape(n_chunks * C, P).T.copy()  # [P, n_chunks*C]
        ei16 = np.concatenate([_wrap16(e) for e in eidx], axis=1)

        glo, ghi = graph_bounds[k], graph_bounds[k + 1]
        ngr = ghi - glo
        assert ngr <= GMAX, ngr
        # S [n_chunks, P, GMAX] one-hot graph membership for local nodes
        gb = np.zeros(n_shard, np.int64)
        gb[:nk] = batch[lo:hi] - glo
        S = np.zeros((n_shard, GMAX), np.float32)
        S[np.arange(nk), gb[:nk]] = 1.0
        S3 = S.reshape(n_chunks, P, GMAX)
        cnts = np.bincount(gb[:nk], minlength=GMAX).astype(np.float32)
        cntinv = (1.0 / np.maximum(cnts, 1.0)).reshape(GMAX, 1).astype(np.float32)
        # AB gather indices: slot (c*128+p) -> graph of node (c, p)
        gbpm = gb.reshape(n_chunks, P)   # [c, p]
        gb16 = _wrap16(gbpm.reshape(-1))

        gnp1 = np.concatenate([
            np.tile(alpha1, (GMAX, 1)), np.tile(weight1, (GMAX, 1)),
            np.tile(bias1, (GMAX, 1))], axis=1).astype(np.float32)
        gnp2 = np.concatenate([
            np.tile(alpha2, (GMAX, 1)), np.tile(weight2, (GMAX, 1)),
            np.tile(bias2, (GMAX, 1))], axis=1).astype(np.float32)

        in_maps.append({
            "xkT": xkT, "W1": np.asarray(W1, np.float32),
            "W2": np.asarray(W2, np.float32),
            "b1r": np.tile(np.asarray(b1, np.float32), (P, 11)),
            "b2r": np.tile(np.asarray(b2, np.float32), (P, 11)),
            "dinvk": np.ascontiguousarray(dinvk), "ident": ident,
            "gnp1": gnp1, "gnp2": gnp2, "cntinv": cntinv,
            "S_in": np.ascontiguousarray(S3),
            "dstv1": np.ascontiguousarray(dstv1),
            "eidx_in": ei16,
            "gb16_in": gb16,
            "iotw_in": iotw,
        })
        pool_maps.append((glo, ghi))

    res = run_bass_kernel_spmd(nc, in_maps, list(range(NCORES)))

    out = np.zeros((NUM_GRAPHS, EMB_DIM), np.float32)
    for k in range(NCORES):
        glo, ghi = pool_maps[k]
        out[glo:ghi] = np.asarray(res.results[k]["pool_out"])[:ghi - glo]
    return out
